# revision 4
# baseline (speedup 1.0000x reference)
"""KoLeo loss kernel v2 for Trainium2 (8 NeuronCores, SPMD), raw Bass.

Math: loss = -mean_i 0.5*log(2 - 2*m_i), m_i = max_{j!=i} <xn_i, xn_j>,
xn = row-normalized input. Each core owns 2048 query rows and receives a
per-core ROTATED copy of x (np.roll by -core*2048) so queries are always
rotated rows 0..2047 == chunks 0..3 of the key stream (one shared program,
per-core differences are data only).

Pipeline per core (32 chunks of 512 rows):
  DMA f32 chunk -> ACT: Square+accum -> ln -> exp(-0.5*ln + ln S) = S/norm
  -> ACT Copy*scale -> xn bf16 -> PE transpose (16x 128x128) -> psum bf16
  -> DVE copy-cast -> xT fp8e4 [128, 4, 16384] (feature-major, scaled by S=16).
Matmul: fp8 DoubleRow, contraction 256 per MM: block (qt, kc) = 2 MMs
  lhsT = xT[:, 2a:2a+2, qt*128:...], rhs = xT[:, 2a:2a+2, kc*512:...]
  -> psum [128 q, 512 k] f32 (dots scaled by S^2=256).
Sweeps: kc groups of 3, qt-major, 6 psum banks (2 qt parities x 3), so the
  stationary operand is reused across 3 MMs (LDWEIGHTS stays hidden).
Drains (static per-block engine map): DVE reduce_max -> bmax slots;
  ACT exp(TP*dot)+accum -> esum slots (log-sum-exp ~ max with ~1e-4 bias).
  Diagonal blocks always DVE: add -512*I first (self-dot suppression).
Final: m = max(max_kc bmax, ln(sum esum)/TP); out = ln(2 - m/128);
  host: loss = -0.5/B * sum(out).  Single ACT table set (square/ln/exp).
"""

import math
import sys

import numpy as np

try:
    import concourse.bass as bass
except ImportError:  # harness may run from a bare directory
    sys.path.insert(0, "/opt/trn_rl_repo")
    import concourse.bass as bass

from concourse import mybir
from concourse.bass_utils import run_bass_kernel_spmd

F32 = mybir.dt.float32
BF16 = mybir.dt.bfloat16
FP8 = mybir.dt.float8e4
AF = mybir.ActivationFunctionType
DR = mybir.MatmulPerfMode.DoubleRow

B = 16384
D = 512
NCORES = 8
Q = B // NCORES      # 2048 query rows per core
NCH = B // 512       # 32 chunks of 512 rows
NQT = Q // 128       # 16 query tiles
S = 16.0             # fp8 scale; dots scale by S^2 = 256
TP = 0.664           # LSE temperature on scaled dots (max arg ~66 < 88)
GROUPS = [tuple(range(g, min(g + 3, NCH))) for g in range(0, NCH, 3)]
NG = len(GROUPS)     # 11 groups (last has 2 kc)


def drain_engine(qt: int, kc: int) -> str:
    if kc == qt // 4:
        return "D"                      # diagonal block: must handle dcorr
    return "A" if (qt + kc) % 2 == 0 else "D"


def _plan_drains():
    """Per-iteration drain plan in sweep emission order.

    Returns (blocks, a_total, d_total) where blocks[bidx] =
    (qt, kc, bank, eng, eng_idx, slot).
    """
    blocks = []
    na = nd = 0
    slots_a = [0] * NQT
    slots_d = [0] * NQT
    for g, kcs in enumerate(GROUPS):
        for qt in range(NQT):
            for i, kc in enumerate(kcs):
                bank = (qt % 2) * 3 + i
                eng = drain_engine(qt, kc)
                if eng == "A":
                    idx, na = na, na + 1
                    slot = slots_a[qt]
                    slots_a[qt] += 1
                else:
                    idx, nd = nd, nd + 1
                    slot = slots_d[qt]
                    slots_d[qt] += 1
                blocks.append((qt, kc, bank, eng, idx, slot))
    return blocks, na, nd


def _build_program(repeat: int = 1, debug: bool = False):
    nc = bass.Bass()
    x = nc.declare_dram_parameter("x", [B, D], F32, isOutput=False)
    ident = nc.declare_dram_parameter("ident", [128, 128], BF16, isOutput=False)
    dcorr = nc.declare_dram_parameter("dcorr", [128, 128], BF16, isOutput=False)
    out = nc.declare_dram_parameter("out", [128, NQT], F32, isOutput=True)
    dbg = {}
    if debug:
        for nm, shape in [("dbmax", [128, NQT, NCH]), ("desum", [128, NQT, NCH]),
                          ("dse", [128, NQT]), ("dlnse", [128, NQT]),
                          ("dm1", [128, NQT]), ("dmfin", [128, NQT]),
                          ("duu", [128, NQT]), ("dxT", [128, 4, 512])]:
            dbg[nm] = nc.declare_dram_parameter(nm, shape, F32 if nm != "dxT" else FP8,
                                                isOutput=True)

    blocks, A_TOT, D_TOT = _plan_drains()
    NBLK = len(blocks)            # 512
    # map bidx -> info; also per-sweep slices
    sweep_blocks = {}
    bidx = 0
    for g, kcs in enumerate(GROUPS):
        lst = []
        for qt in range(NQT):
            for i, kc in enumerate(kcs):
                lst.append((bidx, *blocks[bidx][:]))
                bidx += 1
        sweep_blocks[g] = lst

    from contextlib import ExitStack
    ctx = ExitStack()
    with ctx:
        sb = lambda name, shape, dt: ctx.enter_context(nc.sbuf_tensor(name, shape, dt))
        pt = lambda name, shape, dt: ctx.enter_context(nc.psum_tensor(name, shape, dt))
        sem = lambda name: ctx.enter_context(nc.semaphore(name))

        xT = sb("xT", [128, 4, B], FP8)          # feature-major, S-scaled fp8
        xb = sb("xb", [128, 4, 4, D], F32)       # raw chunks, 4 bufs
        xn = sb("xn", [128, 3, 4, D], BF16)      # normalized*S, 3 bufs
        sqs = sb("sqs", [128, D], BF16)          # Square dump
        ssq = sb("ssq", [128, 4, 4], F32)        # row sumsq
        lnt = sb("lnt", [128, 4], F32)
        rn = sb("rn", [128, 4, 4], F32)          # S/norm
        bmax = sb("bmax", [128, NQT, NCH], F32)
        esum = sb("esum", [128, NQT, NCH], F32)
        m1 = sb("m1", [128, NQT], F32)
        se = sb("se", [128, NQT], F32)
        lnse = sb("lnse", [128, NQT], F32)
        lnse2 = sb("lnse2", [128, NQT], F32)
        dum = sb("dum", [128, 1], F32)           # spacer for DVE RAW hazards
        mfin = sb("mfin", [128, NQT], F32)
        uu = sb("uu", [128, NQT], F32)
        ot = sb("ot", [128, NQT], F32)
        negC = sb("negC", [128, 1], F32)         # exp bias: keep ln(se) in range
        esc = sb("esc", [128, D], F32)           # exp dump
        ident_sb = sb("ident_sb", [128, 128], BF16)
        dcorr_sb = sb("dcorr_sb", [128, 128], BF16)

        ps = [pt(f"psb{i}", [128, 512], F32) for i in range(6)]
        tpp = [pt(f"tpb{i}", [128, 2, 512], BF16) for i in range(2)]

        s_load = sem("s_load")   # DMA: +16 per chunk load
        s_cload = sem("s_cload") # DMA: +16 per const load (ident, dcorr)
        s_xn = sem("s_xn")       # ACT: +1 per chunk normalized (xb consumed)
        s_tp = sem("s_tp")       # PE: +1 per transpose (16/chunk)
        s_tpc = sem("s_tpc")     # DVE: +1 per half-chunk copy (2/chunk)
        s_mm = sem("s_mm")       # PE: +1 per MM block
        s_frA = sem("s_frA")     # ACT drains
        s_frD = sem("s_frD")     # DVE drains
        s_ms = sem("s_ms")       # DVE memsets (2/iter)
        s_se = sem("s_se")       # DVE finals stage1 (1/iter)
        s_ln = sem("s_ln")       # ACT ln(se) (1/iter)
        s_u = sem("s_u")         # DVE finals stage2 (1/iter)
        s_ot = sem("s_ot")       # ACT final ln (1/iter)
        s_rn = sem("s_rn")       # ACT rn ready (1/chunk) — the Copy's scale
                                 # operand is latched at dispatch, so an
                                 # explicit wait must separate producer/consumer

        block = ctx.enter_context(nc.Block())

        LNS = float(math.log(S))

        def chunk_src(c):
            return x[c * 512:(c + 1) * 512, :].rearrange("(j p) d -> p j d", p=128)

        # sweep g is emitted on PE right after transposing chunk EMIT_AT[g]
        EMIT_AT = {g: (3 if g == 0 else GROUPS[g][-1]) for g in range(NG)}
        # chunks normalized on ACT during sweep g (next groups' chunks)
        norm_during = {g: [] for g in range(NG)}
        assigned = 4  # chunks 0..3 normalized in prologue
        for g in range(NG):
            # during sweep g, normalize chunks through 3g+5 (needed by the
            # transposes that precede sweep g+1); norm(3g+5) waits on
            # tp(3g+2) which PE completes before sweep g starts.
            want = min(NCH, GROUPS[g][-1] + 4)
            while assigned < want:
                norm_during[g].append(assigned)
                assigned += 1
        # distribute any leftovers into the earliest groups possible
        assert assigned == NCH

        @block.sync
        def _(sync):
            sync.dma_start(out=ident_sb[:], in_=ident[:]).then_inc(s_cload, 16)
            sync.dma_start(out=dcorr_sb[:], in_=dcorr[:]).then_inc(s_cload, 16)
            for r in range(repeat):
                for c in range(NCH):
                    gi = r * NCH + c
                    if gi >= 4:
                        # xb[c%4] free once chunk gi-4 consumed by ACT
                        sync.wait_ge(s_xn, gi - 3)
                    if gi >= 1:
                        # chain transfers: completion-order == issue-order
                        sync.wait_ge(s_load, 16 * gi)
                    sync.dma_start(out=xb[:, c % 4], in_=chunk_src(c)).then_inc(
                        s_load, 16
                    )
            sync.wait_ge(s_ot, repeat)
            sync.dma_start(out=out[:], in_=ot[:]).then_inc(s_load, 16)
            if debug:
                for nm, src_ap in [("dbmax", bmax[:]), ("desum", esum[:]),
                                   ("dse", se[:]), ("dlnse", lnse[:]),
                                   ("dm1", m1[:]), ("dmfin", mfin[:]),
                                   ("duu", uu[:]),
                                   ("dxT", xT[:, :, 0:512])]:
                    sync.dma_start(out=dbg[nm][:], in_=src_ap).then_inc(s_load, 16)

        # ---------------- ACT ----------------
        def act_norm_chunk(scalar, r, c):
            gi = r * NCH + c
            scalar.wait_ge(s_load, 16 * (gi + 1))
            for j in range(4):
                nc.scalar.activation(
                    out=sqs[:], in_=xb[:, c % 4, j, :], func=AF.Square,
                    accum_out=ssq[:, c % 4, j:j + 1],
                )
            # ln(ssq/S^2) then exp(-0.5*...) = S/norm, batched over j
            nc.scalar.activation(
                out=lnt[:], in_=ssq[:, c % 4, :], func=AF.Ln,
                scale=1.0 / (S * S),
            )
            nc.scalar.activation(
                out=rn[:, c % 4, :], in_=lnt[:], func=AF.Exp,
                scale=-0.5,
            ).then_inc(s_rn, 1)
            scalar.wait_ge(s_rn, gi + 1)
            if gi >= 3:
                # xn[c%3] free once PE transposed chunk gi-3
                scalar.wait_ge(s_tp, 16 * (gi - 2))
            for j in range(4):
                ins = nc.scalar.activation(
                    out=xn[:, c % 3, j, :], in_=xb[:, c % 4, j, :],
                    func=AF.Copy, scale=rn[:, c % 4, j:j + 1],
                )
                if j == 3:
                    ins.then_inc(s_xn, 1)

        def act_drain(scalar, r, bidx, qt, kc, bank, idx, slot):
            scalar.wait_ge(s_mm, r * NBLK + bidx + 1)
            if r > 0 or True:
                pass
            nc.scalar.activation(
                out=esc[:], in_=ps[bank][:], func=AF.Exp, scale=TP,
                bias=negC[:],
                accum_out=esum[:, qt, slot:slot + 1],
            ).then_inc(s_frA, 1)

        @block.scalar
        def _(scalar):
            for r in range(repeat):
                for c in range(4):
                    act_norm_chunk(scalar, r, c)
                for g in range(NG):
                    # interleave this sweep's ACT drains with next chunks' norms
                    todo = [b for b in sweep_blocks[g] if b[4] == "A"]
                    norms = norm_during[g]
                    n_t = len(todo)
                    pts = [
                        int((k + 1) * n_t / (len(norms) + 1)) for k in range(len(norms))
                    ]
                    ni = 0
                    for t, (bidx, qt, kc, bank, eng, idx, slot) in enumerate(todo):
                        # esum slots of iteration r must be zeroed first
                        if g == 0 and t == 0:
                            scalar.wait_ge(s_ms, 3 * (r + 1))
                        act_drain(scalar, r, bidx, qt, kc, bank, idx, slot)
                        while ni < len(norms) and t + 1 == pts[ni]:
                            act_norm_chunk(scalar, r, norms[ni])
                            ni += 1
                    while ni < len(norms):
                        act_norm_chunk(scalar, r, norms[ni])
                        ni += 1
                # finals
                scalar.wait_ge(s_se, r + 1)
                nc.scalar.activation(
                    out=lnse[:], in_=se[:], func=AF.Ln,
                ).then_inc(s_ln, 1)
                scalar.wait_ge(s_u, r + 1)
                nc.scalar.activation(
                    out=ot[:], in_=uu[:], func=AF.Ln,
                ).then_inc(s_ot, 1)

        # ---------------- PE ----------------
        @block.tensor
        def _(tensor):
            tensor.wait_ge(s_cload, 32)  # ident + dcorr
            bank_last = {}  # bank -> (eng, cumulative idx)
            for r in range(repeat):
                for c in range(NCH):
                    gi = r * NCH + c
                    for a1 in range(2):
                        h = 2 * gi + a1
                        if h >= 2:
                            tensor.wait_ge(s_tpc, h - 1)
                        if a1 == 0:
                            tensor.wait_ge(s_xn, gi + 1)
                        for si in range(2):
                            s = 2 * a1 + si
                            for j in range(4):
                                nc.tensor.transpose(
                                    out=tpp[h % 2][:, si, j * 128:(j + 1) * 128],
                                    in_=xn[:, c % 3, j, s * 128:(s + 1) * 128],
                                    identity=ident_sb[:],
                                ).then_inc(s_tp, 1)
                    g_emit = [g for g in range(NG) if EMIT_AT[g] == c]
                    for g in g_emit:
                        kcs = GROUPS[g]
                        tensor.wait_ge(
                            s_tpc, r * 2 * NCH + 2 * (max(3, kcs[-1]) + 1)
                        )
                        for qt in range(NQT):
                            banks = [(qt % 2) * 3 + i for i in range(len(kcs))]
                            for bank in banks:
                                if bank in bank_last:
                                    eng, cidx = bank_last[bank]
                                    tensor.wait_ge(
                                        s_frA if eng == "A" else s_frD, cidx + 1
                                    )
                            for a in range(2):
                                for i, kc in enumerate(kcs):
                                    ins = nc.tensor.matmul(
                                        ps[banks[i]][:],
                                        lhsT=xT[:, 2 * a:2 * a + 2,
                                                qt * 128:(qt + 1) * 128],
                                        rhs=xT[:, 2 * a:2 * a + 2,
                                               kc * 512:(kc + 1) * 512],
                                        start=(a == 0),
                                        stop=(a == 1),
                                        perf_mode=DR,
                                    )
                                    if a == 1:
                                        ins.then_inc(s_mm, 1)
                            # record drains that will free these banks
                            for i, kc in enumerate(kcs):
                                base = next(
                                    b for b in sweep_blocks[g]
                                    if b[1] == qt and b[2] == kc
                                )
                                _, _, _, _, eng, idx, _ = base
                                tot = A_TOT if eng == "A" else D_TOT
                                bank_last[banks[i]] = (eng, r * tot + idx)

        # ---------------- DVE ----------------
        def dve_drain(vector, r, bidx, qt, kc, bank, idx, slot):
            vector.wait_ge(s_mm, r * NBLK + bidx + 1)
            if kc == qt // 4:
                off = (qt % 4) * 128
                nc.vector.tensor_add(
                    out=ps[bank][:, off:off + 128],
                    in0=ps[bank][:, off:off + 128],
                    in1=dcorr_sb[:],
                )
            nc.vector.reduce_max(
                out=bmax[:, qt, slot:slot + 1], in_=ps[bank][:],
                axis=mybir.AxisListType.X,
            ).then_inc(s_frD, 1)

        @block.vector
        def _(vector):
            for r in range(repeat):
                nc.vector.memset(bmax[:], -1e30).then_inc(s_ms, 1)
                nc.vector.memset(esum[:], 0.0).then_inc(s_ms, 1)
                nc.vector.memset(negC[:], -50.0).then_inc(s_ms, 1)
                if r > 0:
                    # xT overwrite must wait until all prev-iter MMs done
                    vector.wait_ge(s_mm, r * NBLK)
                # interleave tp-copies with drains, driven by PE's emit points
                for c in range(NCH):
                    gi = r * NCH + c
                    for a1 in range(2):
                        h = 2 * gi + a1
                        vector.wait_ge(s_tp, 16 * gi + 8 * (a1 + 1))
                        nc.vector.tensor_copy(
                            out=xT[:, 2 * a1:2 * a1 + 2, c * 512:(c + 1) * 512],
                            in_=tpp[h % 2][:, :, :],
                        ).then_inc(s_tpc, 1)
                    for g in [g for g in range(NG) if EMIT_AT[g] == c]:
                        for (bidx, qt, kc, bank, eng, idx, slot) in sweep_blocks[g]:
                            if eng == "D":
                                dve_drain(vector, r, bidx, qt, kc, bank, idx, slot)
                # finals — note: DVE->DVE SBUF RAW at distance 1 is NOT
                # ordered on HW; memset spacers give the writes time to land.
                vector.wait_ge(s_frA, (r + 1) * A_TOT)
                nc.vector.memset(dum[:], 0.0)
                nc.vector.reduce_max(
                    out=m1[:], in_=bmax[:], axis=mybir.AxisListType.X,
                )
                nc.vector.reduce_sum(
                    out=se[:], in_=esum[:], axis=mybir.AxisListType.X,
                ).then_inc(s_se, 1)
                vector.wait_ge(s_ln, r + 1)
                nc.vector.tensor_scalar(
                    out=lnse2[:], in0=lnse[:], scalar1=50.0, scalar2=1.0 / TP,
                    op0=mybir.AluOpType.add, op1=mybir.AluOpType.mult,
                )
                nc.vector.memset(dum[:], 0.0)
                nc.vector.tensor_max(out=mfin[:], in0=lnse2[:], in1=m1[:])
                nc.vector.memset(dum[:], 0.0)
                nc.vector.tensor_scalar(
                    out=uu[:], in0=mfin[:], scalar1=-1.0 / 128.0, scalar2=2.0,
                    op0=mybir.AluOpType.mult, op1=mybir.AluOpType.add,
                ).then_inc(s_u, 1)

    return nc


_NC_CACHE = {}


def _get_program(repeat: int = 1):
    if repeat not in _NC_CACHE:
        _NC_CACHE[repeat] = _build_program(repeat)
    return _NC_CACHE[repeat]


def make_in_maps(x: np.ndarray):
    import ml_dtypes

    x = np.ascontiguousarray(x, dtype=np.float32)
    assert x.shape == (B, D), x.shape
    ident = np.eye(128, dtype=np.float32).astype(ml_dtypes.bfloat16)
    dcorr = (-512.0 * np.eye(128, dtype=np.float32)).astype(ml_dtypes.bfloat16)
    in_maps = []
    for c in range(NCORES):
        in_maps.append({
            "x": np.ascontiguousarray(np.roll(x, -c * Q, axis=0)),
            "ident": ident,
            "dcorr": dcorr,
        })
    return in_maps


def reduce_outputs(results) -> np.ndarray:
    total = 0.0
    for c in range(NCORES):
        total += np.asarray(results[c]["out"], dtype=np.float64).sum()
    return np.array(np.float32(-0.5 * total / B), dtype=np.float32)


def kernel(output: np.ndarray) -> np.ndarray:
    nc = _get_program()
    res = run_bass_kernel_spmd(nc, make_in_maps(output), list(range(NCORES)))
    return reduce_outputs(res.results)


# revision 5
# speedup vs baseline: 1.2657x; 1.2657x over previous
"""KoLeo loss kernel v2 for Trainium2 (8 NeuronCores, SPMD), raw Bass.

Math: loss = -mean_i 0.5*log(2 - 2*m_i), m_i = max_{j!=i} <xn_i, xn_j>,
xn = row-normalized input. Each core owns 2048 query rows and receives a
per-core ROTATED copy of x (np.roll by -core*2048) so queries are always
rotated rows 0..2047 == chunks 0..3 of the key stream (one shared program,
per-core differences are data only).

Pipeline per core (32 chunks of 512 rows):
  DMA f32 chunk -> ACT: Square+accum -> ln -> exp(-0.5*ln + ln S) = S/norm
  -> ACT Copy*scale -> xn bf16 -> PE transpose (16x 128x128) -> psum bf16
  -> DVE copy-cast -> xT fp8e4 [128, 4, 16384] (feature-major, scaled by S=16).
Matmul: fp8 DoubleRow, contraction 256 per MM: block (qt, kc) = 2 MMs
  lhsT = xT[:, 2a:2a+2, qt*128:...], rhs = xT[:, 2a:2a+2, kc*512:...]
  -> psum [128 q, 512 k] f32 (dots scaled by S^2=256).
Sweeps: kc groups of 3, qt-major, 6 psum banks (2 qt parities x 3), so the
  stationary operand is reused across 3 MMs (LDWEIGHTS stays hidden).
Drains (static per-block engine map): DVE reduce_max -> bmax slots;
  ACT exp(TP*dot)+accum -> esum slots (log-sum-exp ~ max with ~1e-4 bias).
  Diagonal blocks always DVE: add -512*I first (self-dot suppression).
Final: m = max(max_kc bmax, ln(sum esum)/TP); out = ln(2 - m/128);
  host: loss = -0.5/B * sum(out).  Single ACT table set (square/ln/exp).
"""

import math
import sys

import numpy as np

try:
    import concourse.bass as bass
except ImportError:  # harness may run from a bare directory
    sys.path.insert(0, "/opt/trn_rl_repo")
    import concourse.bass as bass

from concourse import mybir
from concourse.bass_utils import run_bass_kernel_spmd

F32 = mybir.dt.float32
BF16 = mybir.dt.bfloat16
FP8 = mybir.dt.float8e4
AF = mybir.ActivationFunctionType
DR = mybir.MatmulPerfMode.DoubleRow

B = 16384
D = 512
NCORES = 8
Q = B // NCORES      # 2048 query rows per core
NCH = B // 512       # 32 chunks of 512 rows
NQT = Q // 128       # 16 query tiles
S = 16.0             # fp8 scale; dots scale by S^2 = 256
TP = 0.664           # LSE temperature on scaled dots (max arg ~66 < 88)
GROUPS = [tuple(range(g, min(g + 3, NCH))) for g in range(0, NCH, 3)]
NG = len(GROUPS)     # 11 groups (last has 2 kc)


def drain_engine(qt: int, kc: int) -> str:
    if kc == qt // 4:
        return "D"                      # diagonal block: must handle dcorr
    return "A" if (qt + kc) % 2 == 0 else "D"


def _plan_drains():
    """Per-iteration drain plan in sweep emission order.

    Returns (blocks, a_total, d_total) where blocks[bidx] =
    (qt, kc, bank, eng, eng_idx, slot).
    """
    blocks = []
    na = nd = 0
    slots_a = [0] * NQT
    slots_d = [0] * NQT
    for g, kcs in enumerate(GROUPS):
        for qt in range(NQT):
            for i, kc in enumerate(kcs):
                bank = (qt % 2) * 3 + i
                eng = drain_engine(qt, kc)
                if eng == "A":
                    idx, na = na, na + 1
                    slot = slots_a[qt]
                    slots_a[qt] += 1
                else:
                    idx, nd = nd, nd + 1
                    slot = slots_d[qt]
                    slots_d[qt] += 1
                blocks.append((qt, kc, bank, eng, idx, slot))
    return blocks, na, nd


def _build_program(repeat: int = 1, debug: bool = False):
    nc = bass.Bass()
    x = nc.declare_dram_parameter("x", [B, D], F32, isOutput=False)
    ident = nc.declare_dram_parameter("ident", [128, 128], BF16, isOutput=False)
    dcorr = nc.declare_dram_parameter("dcorr", [128, 128], BF16, isOutput=False)
    out = nc.declare_dram_parameter("out", [128, NQT], F32, isOutput=True)
    dbg = {}
    if debug:
        for nm, shape in [("dbmax", [128, NQT, NCH]), ("desum", [128, NQT, NCH]),
                          ("dse", [128, NQT]), ("dlnse", [128, NQT]),
                          ("dm1", [128, NQT]), ("dmfin", [128, NQT]),
                          ("duu", [128, NQT]), ("dxT", [128, 4, 512])]:
            dbg[nm] = nc.declare_dram_parameter(nm, shape, F32 if nm != "dxT" else FP8,
                                                isOutput=True)

    blocks, A_TOT, D_TOT = _plan_drains()
    NBLK = len(blocks)            # 512
    # map bidx -> info; also per-sweep slices
    sweep_blocks = {}
    bidx = 0
    for g, kcs in enumerate(GROUPS):
        lst = []
        for qt in range(NQT):
            for i, kc in enumerate(kcs):
                lst.append((bidx, *blocks[bidx][:]))
                bidx += 1
        sweep_blocks[g] = lst

    from contextlib import ExitStack
    ctx = ExitStack()
    with ctx:
        sb = lambda name, shape, dt: ctx.enter_context(nc.sbuf_tensor(name, shape, dt))
        pt = lambda name, shape, dt: ctx.enter_context(nc.psum_tensor(name, shape, dt))
        sem = lambda name: ctx.enter_context(nc.semaphore(name))

        xT = sb("xT", [128, 4, B], FP8)          # feature-major, S-scaled fp8
        xb = sb("xb", [128, 4, 4, D], F32)       # raw chunks, 4 bufs
        xn = sb("xn", [128, 3, 4, D], BF16)      # normalized*S, 3 bufs
        sqs = sb("sqs", [128, D], BF16)          # Square dump
        ssq = sb("ssq", [128, 4, 4], F32)        # row sumsq
        lnt = sb("lnt", [128, 4], F32)
        rn = sb("rn", [128, 4, 4], F32)          # S/norm
        bmax = sb("bmax", [128, NQT, NCH], F32)
        esum = sb("esum", [128, NQT, NCH], F32)
        m1 = sb("m1", [128, NQT], F32)
        se = sb("se", [128, NQT], F32)
        lnse = sb("lnse", [128, NQT], F32)
        lnse2 = sb("lnse2", [128, NQT], F32)
        dum = sb("dum", [128, 1], F32)           # spacer for DVE RAW hazards
        mfin = sb("mfin", [128, NQT], F32)
        uu = sb("uu", [128, NQT], F32)
        ot = sb("ot", [128, NQT], F32)
        negC = sb("negC", [128, 1], F32)         # exp bias: keep ln(se) in range
        esc = sb("esc", [128, D], F32)           # exp dump
        ident_sb = sb("ident_sb", [128, 128], BF16)
        dcorr_sb = sb("dcorr_sb", [128, 128], BF16)

        ps = [pt(f"psb{i}", [128, 512], F32) for i in range(6)]
        tpp = [pt(f"tpb{i}", [128, 2, 512], BF16) for i in range(2)]

        s_load = sem("s_load")   # DMA: +16 per chunk load
        s_cload = sem("s_cload") # DMA: +16 per const load (ident, dcorr)
        s_xn = sem("s_xn")       # ACT: +1 per chunk normalized (xb consumed)
        s_tp = sem("s_tp")       # PE: +1 per transpose (16/chunk)
        s_tpc = sem("s_tpc")     # DVE: +1 per half-chunk copy (2/chunk)
        s_mm = sem("s_mm")       # PE: +1 per MM block
        s_frA = sem("s_frA")     # ACT drains
        s_frD = sem("s_frD")     # DVE drains
        s_ms = sem("s_ms")       # DVE memsets (2/iter)
        s_se = sem("s_se")       # DVE finals stage1 (1/iter)
        s_ln = sem("s_ln")       # ACT ln(se) (1/iter)
        s_u = sem("s_u")         # DVE finals stage2 (1/iter)
        s_ot = sem("s_ot")       # ACT final ln (1/iter)
        s_rn = sem("s_rn")       # ACT rn ready (1/chunk) — the Copy's scale
                                 # operand is latched at dispatch, so an
                                 # explicit wait must separate producer/consumer

        block = ctx.enter_context(nc.Block())

        LNS = float(math.log(S))

        def chunk_src(c):
            return x[c * 512:(c + 1) * 512, :].rearrange("(j p) d -> p j d", p=128)

        # sweep g is emitted on PE right after transposing chunk EMIT_AT[g]
        EMIT_AT = {g: min(NCH - 1, GROUPS[g][-1] + 3) for g in range(NG)}
        # chunks normalized on ACT during sweep g (next groups' chunks)
        norm_during = {g: [] for g in range(NG)}
        assigned = 6  # chunks 0..5 normalized in prologue
        for g in range(NG):
            # during segment g, normalize chunks through 3g+8 (their
            # transposes precede sweep g+1, which is one group ahead);
            # norm(3g+8) waits tp(3g+5) which PE completes pre-sweep-g.
            want = min(NCH, GROUPS[g][-1] + 7)
            while assigned < want:
                norm_during[g].append(assigned)
                assigned += 1
        # distribute any leftovers into the earliest groups possible
        assert assigned == NCH

        @block.sync
        def _(sync):
            sync.dma_start(out=ident_sb[:], in_=ident[:]).then_inc(s_cload, 16)
            sync.dma_start(out=dcorr_sb[:], in_=dcorr[:]).then_inc(s_cload, 16)
            for r in range(repeat):
                for c in range(NCH):
                    gi = r * NCH + c
                    if gi >= 4:
                        # xb[c%4] free once chunk gi-4 consumed by ACT
                        sync.wait_ge(s_xn, gi - 3)
                    if gi >= 1:
                        # chain transfers: completion-order == issue-order
                        sync.wait_ge(s_load, 16 * gi)
                    sync.dma_start(out=xb[:, c % 4], in_=chunk_src(c)).then_inc(
                        s_load, 16
                    )
            sync.wait_ge(s_ot, repeat)
            sync.dma_start(out=out[:], in_=ot[:]).then_inc(s_load, 16)
            if debug:
                for nm, src_ap in [("dbmax", bmax[:]), ("desum", esum[:]),
                                   ("dse", se[:]), ("dlnse", lnse[:]),
                                   ("dm1", m1[:]), ("dmfin", mfin[:]),
                                   ("duu", uu[:]),
                                   ("dxT", xT[:, :, 0:512])]:
                    sync.dma_start(out=dbg[nm][:], in_=src_ap).then_inc(s_load, 16)

        # ---------------- ACT ----------------
        def act_norm_chunk(scalar, r, c):
            gi = r * NCH + c
            scalar.wait_ge(s_load, 16 * (gi + 1))
            for j in range(4):
                nc.scalar.activation(
                    out=sqs[:], in_=xb[:, c % 4, j, :], func=AF.Square,
                    accum_out=ssq[:, c % 4, j:j + 1],
                )
            # ln(ssq/S^2) then exp(-0.5*...) = S/norm, batched over j
            nc.scalar.activation(
                out=lnt[:], in_=ssq[:, c % 4, :], func=AF.Ln,
                scale=1.0 / (S * S),
            )
            nc.scalar.activation(
                out=rn[:, c % 4, :], in_=lnt[:], func=AF.Exp,
                scale=-0.5,
            ).then_inc(s_rn, 1)
            scalar.wait_ge(s_rn, gi + 1)
            if gi >= 3:
                # xn[c%3] free once PE transposed chunk gi-3
                scalar.wait_ge(s_tp, 16 * (gi - 2))
            for j in range(4):
                ins = nc.scalar.activation(
                    out=xn[:, c % 3, j, :], in_=xb[:, c % 4, j, :],
                    func=AF.Copy, scale=rn[:, c % 4, j:j + 1],
                )
                if j == 3:
                    ins.then_inc(s_xn, 1)

        def act_drain(scalar, r, bidx, qt, kc, bank, idx, slot):
            scalar.wait_ge(s_mm, r * NBLK + bidx + 1)
            if r > 0 or True:
                pass
            nc.scalar.activation(
                out=esc[:], in_=ps[bank][:], func=AF.Exp, scale=TP,
                bias=negC[:],
                accum_out=esum[:, qt, slot:slot + 1],
            ).then_inc(s_frA, 1)

        @block.scalar
        def _(scalar):
            for r in range(repeat):
                for c in range(6):
                    act_norm_chunk(scalar, r, c)
                for g in range(NG):
                    # interleave this sweep's ACT drains with next chunks' norms
                    todo = [b for b in sweep_blocks[g] if b[4] == "A"]
                    norms = norm_during[g]
                    n_t = len(todo)
                    pts = [
                        int((k + 1) * n_t / (len(norms) + 1)) for k in range(len(norms))
                    ]
                    ni = 0
                    for t, (bidx, qt, kc, bank, eng, idx, slot) in enumerate(todo):
                        # esum slots of iteration r must be zeroed first
                        if g == 0 and t == 0:
                            scalar.wait_ge(s_ms, 3 * (r + 1))
                        act_drain(scalar, r, bidx, qt, kc, bank, idx, slot)
                        while ni < len(norms) and t + 1 == pts[ni]:
                            act_norm_chunk(scalar, r, norms[ni])
                            ni += 1
                    while ni < len(norms):
                        act_norm_chunk(scalar, r, norms[ni])
                        ni += 1
                # finals
                scalar.wait_ge(s_se, r + 1)
                nc.scalar.activation(
                    out=lnse[:], in_=se[:], func=AF.Ln,
                ).then_inc(s_ln, 1)
                scalar.wait_ge(s_u, r + 1)
                nc.scalar.activation(
                    out=ot[:], in_=uu[:], func=AF.Ln,
                ).then_inc(s_ot, 1)

        # ---------------- PE ----------------
        @block.tensor
        def _(tensor):
            tensor.wait_ge(s_cload, 32)  # ident + dcorr
            bank_last = {}  # bank -> (eng, cumulative idx)
            for r in range(repeat):
                for c in range(NCH):
                    gi = r * NCH + c
                    for a1 in range(2):
                        h = 2 * gi + a1
                        if h >= 2:
                            tensor.wait_ge(s_tpc, h - 1)
                        if a1 == 0:
                            tensor.wait_ge(s_xn, gi + 1)
                        for si in range(2):
                            s = 2 * a1 + si
                            for j in range(4):
                                nc.tensor.transpose(
                                    out=tpp[h % 2][:, si, j * 128:(j + 1) * 128],
                                    in_=xn[:, c % 3, j, s * 128:(s + 1) * 128],
                                    identity=ident_sb[:],
                                ).then_inc(s_tp, 1)
                    g_emit = [g for g in range(NG) if EMIT_AT[g] == c]
                    for g in g_emit:
                        kcs = GROUPS[g]
                        tensor.wait_ge(
                            s_tpc, r * 2 * NCH + 2 * (max(3, kcs[-1]) + 1)
                        )
                        for qt in range(NQT):
                            banks = [(qt % 2) * 3 + i for i in range(len(kcs))]
                            for bank in banks:
                                if bank in bank_last:
                                    eng, cidx = bank_last[bank]
                                    tensor.wait_ge(
                                        s_frA if eng == "A" else s_frD, cidx + 1
                                    )
                            for a in range(2):
                                for i, kc in enumerate(kcs):
                                    ins = nc.tensor.matmul(
                                        ps[banks[i]][:],
                                        lhsT=xT[:, 2 * a:2 * a + 2,
                                                qt * 128:(qt + 1) * 128],
                                        rhs=xT[:, 2 * a:2 * a + 2,
                                               kc * 512:(kc + 1) * 512],
                                        start=(a == 0),
                                        stop=(a == 1),
                                        perf_mode=DR,
                                    )
                                    if a == 1:
                                        ins.then_inc(s_mm, 1)
                            # record drains that will free these banks
                            for i, kc in enumerate(kcs):
                                base = next(
                                    b for b in sweep_blocks[g]
                                    if b[1] == qt and b[2] == kc
                                )
                                _, _, _, _, eng, idx, _ = base
                                tot = A_TOT if eng == "A" else D_TOT
                                bank_last[banks[i]] = (eng, r * tot + idx)

        # ---------------- DVE ----------------
        def dve_drain(vector, r, bidx, qt, kc, bank, idx, slot):
            vector.wait_ge(s_mm, r * NBLK + bidx + 1)
            if kc == qt // 4:
                off = (qt % 4) * 128
                nc.vector.tensor_add(
                    out=ps[bank][:, off:off + 128],
                    in0=ps[bank][:, off:off + 128],
                    in1=dcorr_sb[:],
                )
            nc.vector.reduce_max(
                out=bmax[:, qt, slot:slot + 1], in_=ps[bank][:],
                axis=mybir.AxisListType.X,
            ).then_inc(s_frD, 1)

        @block.vector
        def _(vector):
            for r in range(repeat):
                nc.vector.memset(bmax[:], -1e30).then_inc(s_ms, 1)
                nc.vector.memset(esum[:], 0.0).then_inc(s_ms, 1)
                nc.vector.memset(negC[:], -50.0).then_inc(s_ms, 1)
                if r > 0:
                    # xT overwrite must wait until all prev-iter MMs done
                    vector.wait_ge(s_mm, r * NBLK)
                # interleave tp-copies with drains, driven by PE's emit points
                for c in range(NCH):
                    gi = r * NCH + c
                    for a1 in range(2):
                        h = 2 * gi + a1
                        vector.wait_ge(s_tp, 16 * gi + 8 * (a1 + 1))
                        nc.vector.tensor_copy(
                            out=xT[:, 2 * a1:2 * a1 + 2, c * 512:(c + 1) * 512],
                            in_=tpp[h % 2][:, :, :],
                        ).then_inc(s_tpc, 1)
                    for g in [g for g in range(NG) if EMIT_AT[g] == c]:
                        for (bidx, qt, kc, bank, eng, idx, slot) in sweep_blocks[g]:
                            if eng == "D":
                                dve_drain(vector, r, bidx, qt, kc, bank, idx, slot)
                # finals — note: DVE->DVE SBUF RAW at distance 1 is NOT
                # ordered on HW; memset spacers give the writes time to land.
                vector.wait_ge(s_frA, (r + 1) * A_TOT)
                nc.vector.memset(dum[:], 0.0)
                nc.vector.reduce_max(
                    out=m1[:], in_=bmax[:], axis=mybir.AxisListType.X,
                )
                nc.vector.reduce_sum(
                    out=se[:], in_=esum[:], axis=mybir.AxisListType.X,
                ).then_inc(s_se, 1)
                vector.wait_ge(s_ln, r + 1)
                nc.vector.tensor_scalar(
                    out=lnse2[:], in0=lnse[:], scalar1=50.0, scalar2=1.0 / TP,
                    op0=mybir.AluOpType.add, op1=mybir.AluOpType.mult,
                )
                nc.vector.memset(dum[:], 0.0)
                nc.vector.tensor_max(out=mfin[:], in0=lnse2[:], in1=m1[:])
                nc.vector.memset(dum[:], 0.0)
                nc.vector.tensor_scalar(
                    out=uu[:], in0=mfin[:], scalar1=-1.0 / 128.0, scalar2=2.0,
                    op0=mybir.AluOpType.mult, op1=mybir.AluOpType.add,
                ).then_inc(s_u, 1)

    return nc


_NC_CACHE = {}


def _get_program(repeat: int = 1):
    if repeat not in _NC_CACHE:
        _NC_CACHE[repeat] = _build_program(repeat)
    return _NC_CACHE[repeat]


def make_in_maps(x: np.ndarray):
    import ml_dtypes

    x = np.ascontiguousarray(x, dtype=np.float32)
    assert x.shape == (B, D), x.shape
    ident = np.eye(128, dtype=np.float32).astype(ml_dtypes.bfloat16)
    dcorr = (-512.0 * np.eye(128, dtype=np.float32)).astype(ml_dtypes.bfloat16)
    in_maps = []
    for c in range(NCORES):
        in_maps.append({
            "x": np.ascontiguousarray(np.roll(x, -c * Q, axis=0)),
            "ident": ident,
            "dcorr": dcorr,
        })
    return in_maps


def reduce_outputs(results) -> np.ndarray:
    total = 0.0
    for c in range(NCORES):
        total += np.asarray(results[c]["out"], dtype=np.float64).sum()
    return np.array(np.float32(-0.5 * total / B), dtype=np.float32)


def kernel(output: np.ndarray) -> np.ndarray:
    nc = _get_program()
    res = run_bass_kernel_spmd(nc, make_in_maps(output), list(range(NCORES)))
    return reduce_outputs(res.results)
